# revision 4
# baseline (speedup 1.0000x reference)
"""Trainium2 Bass kernel for nn_CrossAttention_88837103550622.

Cross-attention with rotary embeddings, returning (out, attn):
    q = tgt @ wq ; k, v = split(src @ wkv)
    rotary on first 32 head-dims of q, k
    attn = softmax(q k^T / 8) ; out = (attn v) @ wout + bout

Sharding: 8 cores = 2 batches x 4 head-groups (4 heads each).
Data-parallel over batch, tensor-parallel over heads; wout row-sharded
partial outputs are summed on the host.

Everything is computed on-device in float32r (TF32-like PE fast path,
~1e-4 relative rounding on pre-rounded inputs) except the softmax path,
which stays fp32.
"""

import numpy as np

B, N_SRC, N_TGT = 2, 2048, 2048
DIM, HEADS, DIM_HEAD, DIM_ROT = 1024, 16, 64, 32
INNER = HEADS * DIM_HEAD
SCALE = DIM_HEAD**-0.5
N_CORES = 8
HPC = 4  # heads per core
HD = HPC * DIM_HEAD  # 256 per-core inner dim

_PROGRAM_CACHE = {}


def _r32(x):
    """Round fp32 -> float32r-representable (zero low 13 mantissa bits, RN)."""
    x = np.ascontiguousarray(x, dtype=np.float32)
    u = x.view(np.uint32)
    u = (u.astype(np.uint64) + 0x1000) & 0xFFFFE000
    return u.astype(np.uint32).view(np.float32)


def _build_program():
    import concourse.mybir as mybir
    import concourse.tile as tile
    from concourse import bacc

    dt = mybir.dt
    f32, f32r = dt.float32, dt.float32r
    AF = mybir.ActivationFunctionType

    nc = bacc.Bacc("TRN2", target_bir_lowering=False, debug=False, num_devices=N_CORES)

    # ---- DRAM I/O (per-core shard views, host prepares) ----
    tgtT_d = nc.dram_tensor("tgtT", [DIM, N_TGT], f32r, kind="ExternalInput").ap()
    srcT_d = nc.dram_tensor("srcT", [DIM, N_SRC], f32r, kind="ExternalInput").ap()
    wq_d = nc.dram_tensor("wq_s", [DIM, HD], f32r, kind="ExternalInput").ap()
    wk_d = nc.dram_tensor("wk_s", [DIM, HD], f32r, kind="ExternalInput").ap()
    wv_d = nc.dram_tensor("wv_s", [DIM, HD], f32r, kind="ExternalInput").ap()
    wout_d = nc.dram_tensor("wout_s", [HD, DIM], f32r, kind="ExternalInput").ap()
    cos_t_d = nc.dram_tensor("cosF_t", [128, N_TGT], f32r, kind="ExternalInput").ap()
    sin_t_d = nc.dram_tensor("sinF_t", [128, N_TGT], f32r, kind="ExternalInput").ap()
    cos_s_d = nc.dram_tensor("cosF_s", [128, N_SRC], f32r, kind="ExternalInput").ap()
    sin_s_d = nc.dram_tensor("sinF_s", [128, N_SRC], f32r, kind="ExternalInput").ap()
    pmat_d = nc.dram_tensor("pmat", [128, 128], f32r, kind="ExternalInput").ap()
    ident_d = nc.dram_tensor("ident", [128, 128], f32, kind="ExternalInput").ap()

    attn_d = nc.dram_tensor(
        "attn_part", [HPC, N_TGT, N_SRC], f32, kind="ExternalOutput"
    ).ap()
    outp_d = nc.dram_tensor("outp", [N_TGT, DIM], f32, kind="ExternalOutput").ap()

    with tile.TileContext(nc) as tc:
        with (
            tc.tile_pool(name="consts", bufs=1) as consts,
            tc.tile_pool(name="fcpool", bufs=2) as fcpool,
            tc.tile_pool(name="cspool", bufs=1) as cspool,
            tc.tile_pool(name="qkv", bufs=1) as qkv,
            tc.tile_pool(name="ab", bufs=2) as ab,
            tc.tile_pool(name="att", bufs=1) as attp,
            tc.tile_pool(name="smalls", bufs=6) as smalls,
        ):
            # ---- constant loads ----
            wq_sb = consts.tile([128, 8, HD], f32r, tag="w1")
            wk_sb = consts.tile([128, 8, HD], f32r, tag="wk")
            wv_sb = consts.tile([128, 8, HD], f32r, tag="wv")
            nc.sync.dma_start(out=wq_sb, in_=wq_d.rearrange("(c p) n -> p c n", p=128))
            nc.sync.dma_start(out=wk_sb, in_=wk_d.rearrange("(c p) n -> p c n", p=128))
            nc.sync.dma_start(out=wv_sb, in_=wv_d.rearrange("(c p) n -> p c n", p=128))
            pmat_sb = consts.tile([128, 128], f32r, tag="pm")
            ident_sb = consts.tile([128, 128], f32, tag="id")
            nc.sync.dma_start(out=pmat_sb, in_=pmat_d)
            nc.sync.dma_start(out=ident_sb, in_=ident_d)

            qnT = qkv.tile([128, 2, N_TGT], f32r, tag="qn")
            knT = qkv.tile([128, 2, N_SRC], f32r, tag="kn")
            v_sb = qkv.tile([128, HPC, 16, DIM_HEAD], f32r, tag="vsb")

            with tc.tile_pool(name="ps1", bufs=1, space="PSUM") as ps1:

                def project(w_sb, srcT_dram, n_tok, out_cb):
                    """out_cb(hc, psum_tile[128, n_tok]) for hc in 0,1."""
                    p_ps = {
                        hc: ps1.tile([128, n_tok], f32, tag=f"p{hc}", name=f"p_ps{hc}")
                        for hc in (0, 1)
                    }
                    for fc in range(8):
                        t_fc = fcpool.tile([128, n_tok], f32r, tag="fc")
                        nc.sync.dma_start(
                            out=t_fc, in_=srcT_dram[fc * 128 : (fc + 1) * 128, :]
                        )
                        for hc in (0, 1):
                            for nb in range(n_tok // 512):
                                nc.tensor.matmul(
                                    p_ps[hc][:, nb * 512 : (nb + 1) * 512],
                                    w_sb[:, fc, hc * 128 : (hc + 1) * 128],
                                    t_fc[:, nb * 512 : (nb + 1) * 512],
                                    start=(fc == 0),
                                    stop=(fc == 7),
                                )
                    for hc in (0, 1):
                        out_cb(hc, p_ps[hc])

                def rotary(hc, p_ps, cos_sb, sin_sb, outT, n_tok):
                    """outT[:, hc, :] = p*cosF + (P @ p)*sinF."""
                    qp = ab.tile([128, n_tok], f32r, tag="A")
                    nc.vector.tensor_copy(qp, p_ps)
                    r_ps = ps1.tile([128, n_tok], f32, tag="p0")
                    for nb in range(n_tok // 512):
                        nc.tensor.matmul(
                            r_ps[:, nb * 512 : (nb + 1) * 512],
                            pmat_sb,
                            qp[:, nb * 512 : (nb + 1) * 512],
                            start=True,
                            stop=True,
                        )
                    t1 = ab.tile([128, n_tok], f32r, tag="B")
                    nc.vector.tensor_mul(t1, qp, cos_sb)
                    t2 = fcpool.tile([128, n_tok], f32r, tag="fc")
                    nc.vector.tensor_mul(t2, r_ps, sin_sb)
                    nc.vector.tensor_add(outT[:, hc, :], t1, t2)

                # Q path (from tgtT)
                cos_t = cspool.tile([128, N_TGT], f32r, tag="cos")
                sin_t = cspool.tile([128, N_TGT], f32r, tag="sin")
                nc.sync.dma_start(out=cos_t, in_=cos_t_d)
                nc.sync.dma_start(out=sin_t, in_=sin_t_d)
                q_tiles = {}
                project(wq_sb, tgtT_d, N_TGT, lambda hc, ps: q_tiles.update({hc: ps}))
                for hc in (0, 1):
                    rotary(hc, q_tiles[hc], cos_t, sin_t, qnT, N_TGT)

                # K path (from srcT)
                cos_s = cspool.tile([128, N_SRC], f32r, tag="cos")
                sin_s = cspool.tile([128, N_SRC], f32r, tag="sin")
                nc.sync.dma_start(out=cos_s, in_=cos_s_d)
                nc.sync.dma_start(out=sin_s, in_=sin_s_d)
                k_tiles = {}
                project(wk_sb, srcT_d, N_SRC, lambda hc, ps: k_tiles.update({hc: ps}))
                for hc in (0, 1):
                    rotary(hc, k_tiles[hc], cos_s, sin_s, knT, N_SRC)

                # V path: vT then per-head transpose to [j, d]
                vT = qkv.tile([128, 2, N_SRC], f32r, tag="vo")

                def v_cb(hc, ps):
                    nc.vector.tensor_copy(vT[:, hc, :], ps)

                project(wv_sb, srcT_d, N_SRC, v_cb)
                for h in range(HPC):
                    pr, chk = (h % 2) * 64, h // 2
                    v_ps = ps1.tile([128, 1024], f32, tag="p1")
                    for jc in range(16):
                        nc.tensor.transpose(
                            v_ps[:, jc * 64 : (jc + 1) * 64],
                            vT[pr : pr + 64, chk, jc * 128 : (jc + 1) * 128].bitcast(
                                f32
                            ),
                            ident_sb[pr : pr + 64, pr : pr + 64],
                        )
                    nc.vector.tensor_copy(
                        v_sb[:, h, :, :],
                        v_ps.rearrange("p (a b) -> p a b", a=16),
                    )

            # ---- attention phase ----
            out2nT = qkv.tile([128, 2, N_TGT], f32r, tag="vo")
            with (
                tc.tile_pool(name="psS", bufs=1, space="PSUM") as psS,
                tc.tile_pool(name="psT", bufs=1, space="PSUM") as psT,
                tc.tile_pool(name="psO", bufs=2, space="PSUM") as psO,
            ):
                for h in range(HPC):
                    pr, chk = (h % 2) * 64, h // 2
                    for ig in range(4):
                        atT = attp.tile([128, 16, 512], f32r, tag="atT")
                        for it2 in range(4):
                            it = ig * 4 + it2
                            s_ps = psS.tile([128, N_SRC], f32, tag="S")
                            for nb in range(4):
                                nc.tensor.matmul(
                                    s_ps[:, nb * 512 : (nb + 1) * 512],
                                    qnT[pr : pr + 64, chk, it * 128 : (it + 1) * 128],
                                    knT[pr : pr + 64, chk, nb * 512 : (nb + 1) * 512],
                                    start=True,
                                    stop=True,
                                )
                            expS = ab.tile([128, N_SRC], f32, tag="A")
                            sums = smalls.tile([128, 1], f32, tag="sums")
                            nc.scalar.activation(
                                out=expS,
                                in_=s_ps,
                                func=AF.Exp,
                                scale=float(SCALE),
                                accum_out=sums,
                            )
                            inv = smalls.tile([128, 1], f32, tag="inv")
                            nc.vector.reciprocal(inv, sums)
                            attn_t = ab.tile([128, N_SRC], f32, tag="B")
                            nc.vector.tensor_scalar_mul(attn_t, expS, inv)
                            nc.sync.dma_start(
                                out=attn_d[h, it * 128 : (it + 1) * 128, :],
                                in_=attn_t,
                            )
                            for jh in range(2):
                                t_ps = psT.tile([128, 1024], f32, tag="T")
                                for j8 in range(8):
                                    jc = jh * 8 + j8
                                    nc.tensor.transpose(
                                        t_ps[:, j8 * 128 : (j8 + 1) * 128],
                                        attn_t[:, jc * 128 : (jc + 1) * 128],
                                        ident_sb,
                                    )
                                nc.vector.tensor_copy(
                                    atT[
                                        :,
                                        jh * 8 : (jh + 1) * 8,
                                        it2 * 128 : (it2 + 1) * 128,
                                    ],
                                    t_ps.rearrange("p (a b) -> p a b", a=8),
                                )
                        o2 = psO.tile([64, 512], f32, tag="O")
                        for jc in range(16):
                            nc.tensor.matmul(
                                o2,
                                v_sb[:, h, jc, :],
                                atT[:, jc, :],
                                start=(jc == 0),
                                stop=(jc == 15),
                            )
                        nc.vector.tensor_copy(
                            out2nT[pr : pr + 64, chk, ig * 512 : (ig + 1) * 512], o2
                        )

            # ---- output projection (partial; host sums cores + bias) ----
            wout_sb = consts.tile([128, 2, DIM], f32r, tag="w1")
            nc.sync.dma_start(
                out=wout_sb, in_=wout_d.rearrange("(c p) n -> p c n", p=128)
            )
            with tc.tile_pool(name="psP", bufs=2, space="PSUM") as psP:
                for ic in range(16):
                    op_ps = psP.tile([128, DIM], f32, tag="P")
                    for c in range(2):
                        for nb in range(2):
                            nc.tensor.matmul(
                                op_ps[:, nb * 512 : (nb + 1) * 512],
                                out2nT[:, c, ic * 128 : (ic + 1) * 128],
                                wout_sb[:, c, nb * 512 : (nb + 1) * 512],
                                start=(c == 0),
                                stop=(c == 1),
                            )
                    op_sb = ab.tile([128, DIM], f32, tag="B")
                    nc.vector.tensor_copy(op_sb, op_ps)
                    nc.sync.dma_start(
                        out=outp_d[ic * 128 : (ic + 1) * 128, :], in_=op_sb
                    )

    nc.compile()
    return nc


def _get_program():
    if "nc" not in _PROGRAM_CACHE:
        _PROGRAM_CACHE["nc"] = _build_program()
    return _PROGRAM_CACHE["nc"]


def _rotary_layout(cos, sin):
    """cos/sin (n, 32) -> [128, n] tiles matching qT/kT chunk layout.

    Rows [0:32] and [64:96] carry cos^T (the two head-blocks of a chunk);
    rows [32:64], [96:128]: cos->1, sin->0 (pass-through dims).
    """
    n = cos.shape[0]
    cf = np.empty((128, n), np.float32)
    sf = np.empty((128, n), np.float32)
    ct, st = cos.T.astype(np.float32), sin.T.astype(np.float32)
    for blk in (0, 64):
        cf[blk : blk + 32] = ct
        cf[blk + 32 : blk + 64] = 1.0
        sf[blk : blk + 32] = st
        sf[blk + 32 : blk + 64] = 0.0
    return _r32(cf), _r32(sf)


def _pmat():
    """P.T where (P @ x)[2m] = -x[2m+1], (P @ x)[2m+1] = x[2m] on the first
    32 dims of each 64-block; zero elsewhere. Block-diagonal over the two
    64-blocks of a 128-partition chunk."""
    p = np.zeros((128, 128), np.float32)
    for blk in (0, 64):
        for m in range(16):
            p[blk + 2 * m, blk + 2 * m + 1] = -1.0
            p[blk + 2 * m + 1, blk + 2 * m] = 1.0
    return np.ascontiguousarray(p.T)


def kernel(src, sin_src, cos_src, tgt, sin_tgt, cos_tgt, wq, wkv, wout, bout):
    from concourse import bass_utils

    nc = _get_program()

    src = np.asarray(src, np.float32)
    tgt = np.asarray(tgt, np.float32)
    wq = np.asarray(wq, np.float32)
    wkv = np.asarray(wkv, np.float32)
    wout = np.asarray(wout, np.float32)
    bout = np.asarray(bout, np.float32)

    ident = np.eye(128, dtype=np.float32)
    pmat = _pmat()

    in_maps = []
    for c in range(N_CORES):
        b, hg = c // HPC, c % HPC
        hs = hg * HD
        cf_t, sf_t = _rotary_layout(np.asarray(cos_tgt[b]), np.asarray(sin_tgt[b]))
        cf_s, sf_s = _rotary_layout(np.asarray(cos_src[b]), np.asarray(sin_src[b]))
        in_maps.append(
            {
                "tgtT": _r32(tgt[b].T),
                "srcT": _r32(src[b].T),
                "wq_s": _r32(wq[:, hs : hs + HD]),
                "wk_s": _r32(wkv[:, hs : hs + HD]),
                "wv_s": _r32(wkv[:, INNER + hs : INNER + hs + HD]),
                "wout_s": _r32(wout[hs : hs + HD, :]),
                "cosF_t": cf_t,
                "sinF_t": sf_t,
                "cosF_s": cf_s,
                "sinF_s": sf_s,
                "pmat": pmat,
                "ident": ident,
            }
        )

    res = bass_utils.run_bass_kernel_spmd(nc, in_maps, core_ids=list(range(N_CORES)))

    attn = np.empty((B, HEADS, N_TGT, N_SRC), np.float32)
    out = np.zeros((B, N_TGT, DIM), np.float32)
    for c in range(N_CORES):
        b, hg = c // HPC, c % HPC
        r = res.results[c]
        attn[b, hg * HPC : (hg + 1) * HPC] = r["attn_part"]
        out[b] += r["outp"]
    out += bout[None, None, :]
    return out, attn
